# revision 40
# baseline (speedup 1.0000x reference)
"""GAT (2-layer, 4-head) Trainium2 Bass kernel, 8-core SPMD.

Strategy: partition dst nodes across 8 cores by global degree-sorted rank
(rank r -> window r//1024, core (r%1024)//128, lane r%128). Per window, dst =
SBUF partition lane, incoming edges = slots on the free dim with
window-uniform slot counts (shared across cores). Edge source rows are pure
fp16 h vectors (256B) gathered from HBM tables via gpsimd dma_gather with
int16 indices (lo/hi table halves split at 32768), grouped several windows
per gather call.

a_src rides inside the row via a per-head basis change: table rows store
g = h @ M where M = blockdiag_h([w_src_h | orthocomplement]), so
a_src[head] = g[head*32]. The basis is undone by folding M^-1 into the next
layer's matmul (layer 1: explicit M1inv matmul on the idle TensorE before the
ELU; layer 2: folded into Wout on the host). Padding slots gather a sentinel
row whose a_src lanes are -4000, making exp(lrelu(...)) exactly 0 -- no mask
stream at all. Softmax + weighted aggregation run on DVE in fp16 with an
in-place binary-tree slot reduction (contiguous, 2x mode).
"""

import os
import numpy as np
from contextlib import ExitStack

import concourse.bass as bass
import concourse.tile as tile
from concourse import bacc, mybir
from concourse.bass_utils import run_bass_kernel_spmd

# problem constants (hardcoded per contest contract)
N = 50000
E = 1600000
HEADS = 4
HID = 32
INF = 128
OUTF = 8
NCORES = 8
NLOC = N // NCORES            # 6250 dst per core
WPC = (NLOC + 127) // 128     # 49 windows per core
NPAD = WPC * 128              # 6272
SPLIT = 32768                 # int16 gather-index split point
ROWW = 128                    # fp16 words per table row (256 B)
NXP = ((N + 1 + 2047) // 2048) * 2048  # 51200: T1 rows (sentinel + nodes + pad)

# layer-2 table: chunk-major so each chunked AllGather writes contiguously:
# [lo-sentinel | chunk0: 8 cores x 1664 rows | ... | chunk3 | hi-sentinel]
AG_BOUNDS = [13, 26, 39, WPC]            # window boundaries per chunk
AG_ROWS = [128 * b - 128 * a for a, b in
           zip([0] + AG_BOUNDS[:-1], AG_BOUNDS)]   # rows per core per chunk
AG_OFF = [1]
for _r in AG_ROWS:
    AG_OFF.append(AG_OFF[-1] + NCORES * _r)
SENT_HI2 = AG_OFF[-1]                     # 50177
TB2 = SENT_HI2 + 1                        # 50178 rows in layer-2 table


def _t2row(core_n, pos_n):
    """flat T2 row of node owned by core c at per-core position p."""
    w = pos_n // 128
    k = np.searchsorted(np.asarray(AG_BOUNDS), w, side="right")
    start = np.r_[0, 128 * np.asarray(AG_BOUNDS[:-1])]
    off = np.asarray(AG_OFF[:-1])
    rows = np.asarray(AG_ROWS)
    return off[k] + core_n * rows[k] + (pos_n - start[k])
SENT = -4000.0                # sentinel a_src value: exp(lrelu(x+SENT)) == 0
GCAP = int(os.environ.get("GAT_GCAP", "112"))   # max slots per gather group
GWIN = int(os.environ.get("GAT_GWIN", "16"))    # max windows per gather group
QALT = bool(int(os.environ.get("GAT_QALT", "0")))
SPKT = bool(int(os.environ.get("GAT_SP", "0")))

F32 = mybir.dt.float32
F16 = mybir.dt.float16
I16 = mybir.dt.int16

_CACHE = {}
LAST_RESULT = None
LAST_NC = None
LAST_IN_MAPS = None


# ----------------------------------------------------------------------------
# host-side graph preprocessing
# ----------------------------------------------------------------------------

def _host_prep(edge_index):
    srcs = np.concatenate([edge_index[0], np.arange(N)]).astype(np.int64)
    dsts = np.concatenate([edge_index[1], np.arange(N)]).astype(np.int64)
    ne = srcs.shape[0]

    # layer-1 table halves are split by row id (node+1); degree per half
    row1 = srcs + 1
    lo1 = row1 < SPLIT
    deg_lo = np.bincount(dsts[lo1], minlength=N)
    deg_hi = np.bincount(dsts[~lo1], minlength=N)

    # global degree-sorted assignment: rank -> (window, core, lane)
    order = np.lexsort((-deg_hi, -(deg_lo // 6)))
    rank = np.empty(N, np.int64)
    rank[order] = np.arange(N)
    w_n = rank // (128 * NCORES)
    core_n = (rank % (128 * NCORES)) // 128
    lane_n = rank % 128
    pos_n = w_n * 128 + lane_n            # slot of node within its owner core

    # node_of[c, pos] = node id owned by core c at position pos (-1 = pad)
    node_of = np.full((NCORES, NPAD), -1, np.int64)
    node_of[core_n, pos_n] = np.arange(N)

    core = core_n[dsts]
    w_e = w_n[dsts]
    dpart_e = lane_n[dsts]

    t2row = _t2row(core_n, pos_n)

    layers = {}
    for L, (rows, sent_lo, sent_hi) in enumerate([
        (row1, 0, NXP - 1 - SPLIT),
        (t2row[srcs], 0, SENT_HI2 - SPLIT),
    ], start=1):
        sec = (rows >= SPLIT).astype(np.int64)
        val = np.where(rows < SPLIT, rows, rows - SPLIT)

        key = ((core * WPC + w_e) * 2 + sec) * 128 + dpart_e
        order_e = np.argsort(key, kind="stable")
        ks = key[order_e]
        change = np.r_[True, ks[1:] != ks[:-1]]
        gid = np.cumsum(change) - 1
        startpos = np.flatnonzero(change)
        j_sorted = np.arange(ne) - startpos[gid]
        j = np.empty(ne, np.int64)
        j[order_e] = j_sorted

        cnt = np.bincount(key, minlength=NCORES * WPC * 2 * 128)
        cnt = cnt.reshape(NCORES, WPC, 2, 128)
        dsec = cnt.max(axis=(0, 3))          # [WPC, 2]
        dlo = dsec[:, 0].astype(np.int64)
        dhi = dsec[:, 1].astype(np.int64)
        cs_lo = np.r_[0, np.cumsum(dlo)]
        cs_hi = np.r_[0, np.cumsum(dhi)]
        tot_lo, tot_hi = int(cs_lo[-1]), int(cs_hi[-1])

        # unfilled slots gather the sentinel row of the matching half
        idx_lo = np.full((NCORES, max(tot_lo, 1) * 128), sent_lo, np.int16)
        idx_hi = np.full((NCORES, max(tot_hi, 1) * 128), sent_hi, np.int16)

        is_lo = sec == 0
        fpos_lo = (cs_lo[w_e[is_lo]] + j[is_lo]) * 128 + dpart_e[is_lo]
        idx_lo[core[is_lo], fpos_lo] = val[is_lo].astype(np.int16)
        fpos_hi = (cs_hi[w_e[~is_lo]] + j[~is_lo]) * 128 + dpart_e[~is_lo]
        idx_hi[core[~is_lo], fpos_hi] = val[~is_lo].astype(np.int16)

        # rewrap idx arrays into dma_gather layout [128, 8*tot]
        def wrap(arr, dvec, csvec, tot, fill):
            out = np.full((NCORES, 128, 8 * max(tot, 1)), fill, np.int16)
            for w in range(WPC):
                d = int(dvec[w])
                if d == 0:
                    continue
                cs = int(csvec[w])
                blk = arr[:, cs * 128:(cs + d) * 128]          # [NC, d*128]
                blk = blk.reshape(NCORES, -1, 16).transpose(0, 2, 1)  # [NC,16,8d]
                out[:, :, 8 * cs: 8 * (cs + d)] = np.tile(blk, (1, 8, 1))
            return out

        # gather groups: consecutive windows, capped slots/windows per group
        groups = []
        g0 = 0
        acc = 0
        for w in range(WPC):
            d = int(dlo[w] + dhi[w])
            if w > g0 and (acc + d > GCAP or w - g0 >= GWIN):
                groups.append((g0, w))
                g0 = w
                acc = 0
            acc += d
        groups.append((g0, WPC))

        layers[L] = dict(
            dlo=dlo, dhi=dhi, cs_lo=cs_lo, cs_hi=cs_hi,
            tot_lo=tot_lo, tot_hi=tot_hi, groups=groups,
            idx_lo=wrap(idx_lo, dlo, cs_lo, tot_lo, sent_lo),
            idx_hi=wrap(idx_hi, dhi, cs_hi, tot_hi, sent_hi),
        )

    return layers, node_of


def _basis(att_src):
    """M = blockdiag_h([w_h | orthonormal complement]): a_src[h] = (x@M)[h*32]."""
    M = np.zeros((INF, INF), np.float64)
    for h in range(HEADS):
        w = att_src[h].astype(np.float64)          # [32]
        A = np.concatenate([w[:, None], np.eye(HID)], axis=1)
        q, _ = np.linalg.qr(A)                      # q[:,0] = w/|w|
        Mh = np.concatenate([w[:, None], q[:, 1:HID]], axis=1)
        M[h * HID:(h + 1) * HID, h * HID:(h + 1) * HID] = Mh
    return M, np.linalg.inv(M)


# device feature order is head-interleaved: dev col k = c*HEADS + h <-> ref
# col h*HID + c, so a_src sits at dev cols 0..3 and every per-head broadcast
# on DVE has a step-1 innermost axis (2x packed mode)
REF_OF_DEV = np.array([(k % HEADS) * HID + k // HEADS for k in range(INF)])


def _vecdst(att_dst):
    V = np.zeros((INF, HEADS), np.float64)
    for h in range(HEADS):
        V[h * HID:(h + 1) * HID, h] = att_dst[h]
    return V


def _fold_weights(W1, att_src1, att_dst1, b1, W2, att_src2, att_dst2, b2,
                  Wout, bout):
    M1, M1i = _basis(att_src1)
    M2, M2i = _basis(att_src2)
    W1 = W1.astype(np.float64)
    W2 = W2.astype(np.float64)
    p = REF_OF_DEV
    rhs1 = (W1 @ M1)[:, p].astype(np.float16)                   # [128,128]
    wdx1 = (W1 @ _vecdst(att_dst1)).astype(np.float16)          # [128,4]
    m1inv = M1i[p][:, p].astype(np.float16)                     # [128,128]
    rhs2 = np.concatenate([(W2 @ M2)[p][:, p],
                           (W2 @ _vecdst(att_dst2))[p]],
                          axis=1).astype(np.float16)            # [128,132]
    woutm = (M2i @ Wout.astype(np.float64))[p].astype(np.float16)  # [128,8]
    b1t = np.tile(b1.astype(np.float32)[p], (128, 1))           # [128,128]
    bf = (b2 @ Wout + bout).astype(np.float32)
    bft = np.tile(bf, (128, 1))                                 # [128,8]
    return rhs1, wdx1, m1inv, rhs2, woutm, b1t, bft


def _sentrow():
    s = np.zeros((128, ROWW), np.float16)
    s[:, 0:HEADS] = SENT                 # dev cols 0..3 are the a_src slots
    return s


# ----------------------------------------------------------------------------
# device program
# ----------------------------------------------------------------------------

def _edge_phase(nc, tc, pools, meta, table, idx_in, ad_tile, den_tile,
                aggn_cb, tag, after_window=None):
    """Grouped gather + per-window softmax + fp16 tree aggregation.
    aggn_cb(w, aggn_ap) consumes the normalized [128,128] fp16 aggregate;
    after_window(w) is called once window w is fully emitted."""
    dlo, dhi = meta["dlo"], meta["dhi"]
    cs_lo, cs_hi = meta["cs_lo"], meta["cs_hi"]
    groups = meta["groups"]
    gpool, wpool, spool, mpool = pools["g"], pools["w"], pools["s"], pools["m"]
    idx_lo_in, idx_hi_in = idx_in
    qn = 0

    for gi, (g0, g1) in enumerate(groups):
        nlo = int(cs_lo[g1] - cs_lo[g0])
        nhi = int(cs_hi[g1] - cs_hi[g0])

        ilo_g = mpool.tile([128, 8 * max(nlo, 1)], I16, tag="ilog")
        if nlo:
            nc.sync.dma_start(ilo_g[:, 0:8 * nlo],
                              idx_lo_in[:, 8 * int(cs_lo[g0]):8 * int(cs_lo[g1])])
        ihi_g = mpool.tile([128, 8 * max(nhi, 1)], I16, tag="ihig")
        if nhi:
            nc.sync.dma_start(ihi_g[:, 0:8 * nhi],
                              idx_hi_in[:, 8 * int(cs_hi[g0]):8 * int(cs_hi[g1])])

        xg_lo = gpool.tile([128, max(nlo, 1), ROWW], F16, tag="xglo")
        if nlo:
            nc.gpsimd.dma_gather(xg_lo[:], table[0:SPLIT, :], ilo_g[:, 0:8 * nlo],
                                 128 * nlo, 128 * nlo, ROWW,
                                 single_packet=SPKT, queue_num=qn)
            if QALT:
                qn = 1 - qn
        xg_hi = gpool.tile([128, max(nhi, 1), ROWW], F16, tag="xghi")
        if nhi:
            nc.gpsimd.dma_gather(xg_hi[:], table[SPLIT:, :], ihi_g[:, 0:8 * nhi],
                                 128 * nhi, 128 * nhi, ROWW,
                                 single_packet=SPKT, queue_num=qn)
            if QALT:
                qn = 1 - qn

        for w in range(g0, g1):
            d_lo, d_hi = int(dlo[w]), int(dhi[w])
            assert d_lo + d_hi >= 1
            adw = ad_tile[:, 4 * w:4 * w + 4]

            pieces = []
            for sec, dp, xgt, csv in (
                (0, d_lo, xg_lo, int(cs_lo[w] - cs_lo[g0])),
                (1, d_hi, xg_hi, int(cs_hi[w] - cs_hi[g0])),
            ):
                if dp == 0:
                    continue
                xv = xgt[:, csv:csv + dp, :]                     # [128,dp,128] f16
                as4 = xv[:, :, 0:HEADS]                          # [128,dp,4] f16
                lp = spool.tile([128, dp, 4], F16, tag=f"lp{sec}")
                nc.vector.tensor_add(
                    lp[:], as4,
                    adw.unsqueeze(1).broadcast_to([128, dp, 4]))
                ll = spool.tile([128, dp, 4], F16, tag=f"ll{sec}")
                nc.vector.scalar_tensor_tensor(
                    ll[:], lp[:], 0.2, lp[:],
                    mybir.AluOpType.mult, mybir.AluOpType.max)
                ew = spool.tile([128, dp, 4], F16, tag=f"ew{sec}")
                nc.scalar.activation(ew[:], ll[:], mybir.ActivationFunctionType.Exp)

                dn = spool.tile([128, 4], F32, tag=f"dn{sec}")
                nc.vector.tensor_reduce(dn[:], ew[:].transpose([0, 2, 1]),
                                        mybir.AxisListType.X, mybir.AluOpType.add)

                wm = wpool.tile([128, dp, ROWW], F16, tag=f"wm{sec}")
                nc.vector.tensor_mul(
                    wm[:].rearrange("p j (c h) -> p j c h", h=HEADS),
                    xv.rearrange("p j (c h) -> p j c h", h=HEADS),
                    ew[:].unsqueeze(2).broadcast_to([128, dp, HID, HEADS]))
                cur = dp
                while cur > 1:
                    half = cur // 2
                    nc.vector.tensor_add(wm[:, 0:half, :], wm[:, 0:half, :],
                                         wm[:, cur - half:cur, :])
                    cur -= half
                pieces.append((wm, dn))

            # combine pieces, normalize
            den4 = den_tile[:, 4 * w:4 * w + 4]
            agg = spool.tile([128, ROWW], F16, tag="agg")
            if len(pieces) == 2:
                (wl, dl), (wh, dh) = pieces
                nc.vector.tensor_add(den4, dl[:], dh[:])
                nc.vector.tensor_add(agg[:], wl[:, 0:1, :].squeeze(1),
                                     wh[:, 0:1, :].squeeze(1))
            else:
                (wl, dl), = pieces
                nc.vector.tensor_copy(den4, dl[:])
                nc.vector.tensor_copy(agg[:], wl[:, 0:1, :].squeeze(1))
            rec = spool.tile([128, 4], F16, tag="rec")
            with nc.allow_low_precision(reason="softmax denom >= 1; fp16 rec"):
                nc.vector.reciprocal(rec[:], den4)
            aggn = spool.tile([128, ROWW], F16, tag="aggn")
            nc.vector.tensor_mul(
                aggn[:].rearrange("p (c h) -> p c h", h=HEADS),
                agg[:].rearrange("p (c h) -> p c h", h=HEADS),
                rec[:].unsqueeze(1).broadcast_to([128, HID, HEADS]))
            aggn_cb(w, aggn)
            if after_window is not None:
                after_window(w)


def _build_program(meta1, meta2):
    nc = bacc.Bacc("TRN2", num_devices=NCORES,
                   num_swdge_queues=2 if QALT else 1)

    xT = nc.dram_tensor("xT", [128, NXP], F16, kind="ExternalInput")
    xs = nc.dram_tensor("xs", [128, NPAD], F16, kind="ExternalInput")
    rhs1_h = nc.dram_tensor("rhs1", [128, 128], F16, kind="ExternalInput")
    wdx1_h = nc.dram_tensor("wdx1", [128, 4], F16, kind="ExternalInput")
    m1inv_h = nc.dram_tensor("m1inv", [128, 128], F16, kind="ExternalInput")
    rhs2_h = nc.dram_tensor("rhs2", [128, 132], F16, kind="ExternalInput")
    woutm_h = nc.dram_tensor("woutm", [128, 8], F16, kind="ExternalInput")
    b1t_h = nc.dram_tensor("b1t", [128, 128], F32, kind="ExternalInput")
    bft_h = nc.dram_tensor("bft", [128, 8], F32, kind="ExternalInput")
    ident_h = nc.dram_tensor("ident", [128, 128], F16, kind="ExternalInput")
    sent_h = nc.dram_tensor("sentrow", [128, ROWW], F16, kind="ExternalInput")

    i1lo = nc.dram_tensor("i1lo", [128, 8 * max(meta1["tot_lo"], 1)], I16, kind="ExternalInput")
    i1hi = nc.dram_tensor("i1hi", [128, 8 * max(meta1["tot_hi"], 1)], I16, kind="ExternalInput")
    i2lo = nc.dram_tensor("i2lo", [128, 8 * max(meta2["tot_lo"], 1)], I16, kind="ExternalInput")
    i2hi = nc.dram_tensor("i2hi", [128, 8 * max(meta2["tot_hi"], 1)], I16, kind="ExternalInput")

    outy = nc.dram_tensor("outy", [NPAD, OUTF], F32, kind="ExternalOutput")

    T1 = nc.dram_tensor("T1", [NXP, ROWW], F16, kind="Internal")
    AGIN = nc.dram_tensor("AGIN", [NPAD, ROWW], F16, kind="Internal")
    T2 = nc.dram_tensor("T2", [TB2, ROWW], F16, kind="Internal")

    with ExitStack() as ctx:
        tc = ctx.enter_context(tile.TileContext(nc))
        cpool = ctx.enter_context(tc.tile_pool(name="consts", bufs=1))
        pers = ctx.enter_context(tc.tile_pool(name="pers", bufs=1))
        gpool = ctx.enter_context(tc.tile_pool(name="gather", bufs=2))
        wpool = ctx.enter_context(tc.tile_pool(name="work", bufs=2))
        spool = ctx.enter_context(tc.tile_pool(name="small", bufs=3))
        mpool = ctx.enter_context(tc.tile_pool(name="meta", bufs=2))
        pspool = ctx.enter_context(tc.tile_pool(name="ps", bufs=2, space="PSUM"))
        ptpool = ctx.enter_context(tc.tile_pool(name="pt", bufs=2, space="PSUM"))
        pools = {"g": gpool, "w": wpool, "s": spool, "m": mpool}

        def const(h, shape, dtype=F16, tag=None):
            t = cpool.tile(shape, dtype, tag=tag)
            nc.sync.dma_start(t[:], h[:])
            return t

        rhs1_t = const(rhs1_h, [128, 128], tag="rhs1")
        wdx1_t = const(wdx1_h, [128, 4], tag="wdx1")
        m1inv_t = const(m1inv_h, [128, 128], tag="m1inv")
        rhs2_t = const(rhs2_h, [128, 132], tag="rhs2")
        woutm_t = const(woutm_h, [128, 8], tag="woutm")
        b1t_t = const(b1t_h, [128, 128], F32, tag="b1t")
        bft_t = const(bft_h, [128, 8], F32, tag="bft")
        ident_t = const(ident_h, [128, 128], tag="identc")
        sent_t = const(sent_h, [128, ROWW], tag="sentc")
        xs_t = const(xs, [128, NPAD], tag="xs")

        ad1 = pers.tile([128, 4 * WPC], F16)
        ad2 = pers.tile([128, 4 * WPC], F16)
        den1 = pers.tile([128, 4 * WPC], F32)
        den2 = pers.tile([128, 4 * WPC], F32)
        fin = pers.tile([128, OUTF * WPC], F32)

        # ---- phase A: T1[r] = g1 fp16 of node r-1 (row 0/pad = sentinel) ----
        XBLK = 2048
        for b0 in range(0, NXP, XBLK):
            xt_t = gpool.tile([128, XBLK], F16, tag="xglo")
            nc.sync.dma_start(xt_t[:], xT[:, b0:b0 + XBLK])
            rb = wpool.tile([128, XBLK], F16, tag="wm0")
            for k in range(XBLK // 128):
                ps = pspool.tile([128, 132], F32, tag="mm")
                nc.tensor.matmul(ps[:, 0:128], xt_t[:, k * 128:(k + 1) * 128],
                                 rhs1_t[:], start=True, stop=True)
                nc.scalar.copy(rb[:, k * 128:(k + 1) * 128], ps[:, 0:128])
            nc.sync.dma_start(
                T1[b0:b0 + XBLK, :].rearrange("(a p) r -> p a r", p=128),
                rb[:].rearrange("p (a r) -> p a r", a=XBLK // 128))
            if b0 == 0:
                # patch lo sentinel early: lo gathers then only depend on
                # the lo half of the table build
                nc.sync.dma_start(T1[0:1, :], sent_t[0:1, :])
        nc.sync.dma_start(T1[NXP - 1:NXP, :], sent_t[0:1, :])

        # a_d1 for owned (globally sorted) nodes
        for w in range(WPC):
            ps4 = ptpool.tile([128, 4], F32, tag="pss")
            nc.tensor.matmul(ps4[:], xs_t[:, w * 128:(w + 1) * 128], wdx1_t[:],
                             start=True, stop=True)
            nc.vector.tensor_copy(ad1[:, 4 * w:4 * w + 4], ps4[:])

        stop = os.environ.get("GAT_STOP", "full")
        if stop == "a":
            nc.vector.memset(fin[:], 0.0)

        # ---- phase B: layer-1 edge phase + layer-2 row build ----
        def tail1(w, aggn):
            gt_ps = ptpool.tile([128, 128], F16, tag="tr")
            nc.tensor.transpose(gt_ps[:], aggn[:], ident_t[:])
            gt = spool.tile([128, 128], F16, tag="t1gt")
            nc.scalar.copy(gt[:], gt_ps[:])
            ps_h = pspool.tile([128, 132], F32, tag="mm")
            nc.tensor.matmul(ps_h[:, 0:128], gt[:], m1inv_t[:], start=True,
                             stop=True)
            t = spool.tile([128, 128], F32, tag="t1t")
            nc.vector.tensor_add(t[:], ps_h[:, 0:128], b1t_t[:])
            mn = spool.tile([128, 128], F32, tag="t1m")
            nc.vector.tensor_scalar_min(mn[:], t[:], 0.0)
            ex = spool.tile([128, 128], F32, tag="t1e")
            nc.scalar.activation(ex[:], mn[:], mybir.ActivationFunctionType.Exp)
            x2 = spool.tile([128, 128], F32, tag="t1x")
            nc.vector.scalar_tensor_tensor(x2[:], t[:], 0.0, ex[:],
                                           mybir.AluOpType.max, mybir.AluOpType.add)
            x2h = spool.tile([128, 128], F16, tag="t1xh")
            nc.vector.tensor_scalar_sub(x2h[:], x2[:], 1.0)
            x2t_ps = ptpool.tile([128, 128], F16, tag="tr")
            nc.tensor.transpose(x2t_ps[:], x2h[:], ident_t[:])
            x2t = spool.tile([128, 128], F16, tag="t1xt")
            nc.scalar.copy(x2t[:], x2t_ps[:])
            ps2 = pspool.tile([128, 132], F32, tag="mm")
            nc.tensor.matmul(ps2[:], x2t[:], rhs2_t[:], start=True, stop=True)
            rowt = spool.tile([128, ROWW], F16, tag="rowt")
            nc.scalar.copy(rowt[:], ps2[:, 0:128])
            nc.vector.tensor_copy(ad2[:, 4 * w:4 * w + 4], ps2[:, 128:132])
            nc.sync.dma_start(
                AGIN[w * 128:(w + 1) * 128, :].rearrange("(a p) r -> p a r", p=128),
                rowt[:].rearrange("p (a r) -> p a r", a=1))

        # T2 sentinel rows live outside every collective's output range, so
        # they can be patched up front with no ordering constraints
        nc.sync.dma_start(T2[0:1, :], sent_t[0:1, :])
        nc.sync.dma_start(T2[SENT_HI2:SENT_HI2 + 1, :], sent_t[0:1, :])

        # chunked AllGather: each chunk ships as soon as its windows' AGIN
        # rows are written, overlapping the collective with layer-1 tail work
        do_ag = stop not in ("l1", "a")

        def after_w1(w):
            if not do_ag or (w + 1) not in AG_BOUNDS:
                return
            ci = AG_BOUNDS.index(w + 1)
            a = 128 * (0 if ci == 0 else AG_BOUNDS[ci - 1])
            b = 128 * AG_BOUNDS[ci]
            nc.gpsimd.collective_compute(
                "AllGather", mybir.AluOpType.bypass,
                replica_groups=[list(range(NCORES))],
                ins=[AGIN[a:b, :].opt()],
                outs=[T2[AG_OFF[ci]:AG_OFF[ci] + NCORES * AG_ROWS[ci], :].opt()])

        if stop != "a":
            _edge_phase(nc, tc, pools, meta1, T1, (i1lo, i1hi), ad1, den1,
                        tail1, "l1", after_window=after_w1)
        if stop == "l1":
            nc.vector.memset(fin[:], 0.0)

        # ---- phase C: layer-2 edge phase + final projection ----
        def tail2(w, aggn):
            at_ps = ptpool.tile([128, 128], F16, tag="tr")
            nc.tensor.transpose(at_ps[:], aggn[:], ident_t[:])
            at = spool.tile([128, 128], F16, tag="t2at")
            nc.scalar.copy(at[:], at_ps[:])
            ps8 = ptpool.tile([128, 8], F32, tag="pss")
            nc.tensor.matmul(ps8[:], at[:], woutm_t[:], start=True, stop=True)
            nc.vector.tensor_add(fin[:, OUTF * w:OUTF * (w + 1)], ps8[:], bft_t[:])

        if stop == "ag":
            nc.vector.memset(fin[:], 0.0)
        if stop == "full":
            _edge_phase(nc, tc, pools, meta2, T2, (i2lo, i2hi), ad2, den2,
                        tail2, "l2")

        nc.sync.dma_start(outy[:].rearrange("(a p) r -> p a r", p=128),
                          fin[:].rearrange("p (a r) -> p a r", a=WPC))

    nc.compile()
    return nc


# ----------------------------------------------------------------------------
# entry point
# ----------------------------------------------------------------------------

def kernel(x, edge_index, W1, att_src1, att_dst1, b1, W2, att_src2, att_dst2,
           b2, Wout, bout):
    global LAST_RESULT, LAST_NC, LAST_IN_MAPS
    x = np.asarray(x, np.float32)
    edge_index = np.asarray(edge_index)

    ck = hash(edge_index.tobytes())
    if ck not in _CACHE:
        layers, node_of = _host_prep(edge_index)
        nc = _build_program(layers[1], layers[2])
        _CACHE.clear()
        _CACHE[ck] = (layers, node_of, nc)
    layers, node_of, nc = _CACHE[ck]
    meta1, meta2 = layers[1], layers[2]

    rhs1, wdx1, m1inv, rhs2, woutm, b1t, bft = _fold_weights(
        np.asarray(W1, np.float64), np.asarray(att_src1, np.float64),
        np.asarray(att_dst1, np.float64), np.asarray(b1, np.float32),
        np.asarray(W2, np.float64), np.asarray(att_src2, np.float64),
        np.asarray(att_dst2, np.float64), np.asarray(b2, np.float32),
        np.asarray(Wout, np.float64), np.asarray(bout, np.float32))

    xT = np.zeros((128, NXP), np.float16)
    xT[:, 1:N + 1] = x.T.astype(np.float16)
    ident = np.eye(128, dtype=np.float16)
    sent = _sentrow()

    in_maps = []
    for c in range(NCORES):
        xsc = np.zeros((128, NPAD), np.float16)
        own = node_of[c]
        m = own >= 0
        xsc[:, m] = x[own[m]].T.astype(np.float16)
        in_maps.append({
            "xT": xT, "xs": xsc, "rhs1": rhs1, "wdx1": wdx1, "m1inv": m1inv,
            "rhs2": rhs2, "woutm": woutm, "b1t": b1t, "bft": bft,
            "ident": ident, "sentrow": sent,
            "i1lo": np.ascontiguousarray(meta1["idx_lo"][c]),
            "i1hi": np.ascontiguousarray(meta1["idx_hi"][c]),
            "i2lo": np.ascontiguousarray(meta2["idx_lo"][c]),
            "i2hi": np.ascontiguousarray(meta2["idx_hi"][c]),
        })

    trace = bool(int(os.environ.get("GAT_TRACE", "0")))
    res = run_bass_kernel_spmd(nc, in_maps, core_ids=list(range(NCORES)),
                               trace=trace)
    LAST_RESULT = res
    LAST_NC, LAST_IN_MAPS = nc, in_maps

    out = np.empty((N, OUTF), np.float32)
    for c in range(NCORES):
        own = node_of[c]
        m = own >= 0
        out[own[m]] = res.results[c]["outy"][m.nonzero()[0]]
    return out
